# revision 30
# baseline (speedup 1.0000x reference)
"""Trainium2 Bass kernel for MDMLPPatch (3x3 unfold + per-channel linear 9->64).

out[n,c,p,e] = sum_d patches[n,c,p,d] * W[d,e] + b[e]
x: [16,64,56,56] f32, W: [9,64] f32, b: [64] f32 -> out: [16,64,3136,64] f32

Sharding: data-parallel over batch N: 16 n / 8 cores = 2 n per core.
Each core processes 128 independent 56x56 images (2 n x 64 c).

bf16 design (rel tol is 2e-2; bf16 keeps rel err ~5e-3):
  - Host ships S[img, d, u] = unfolded patches in u-order, bf16. d=0..8 are
    the 9 taps, d=9 is all-ones so the K=10 matmul contraction adds the bias.
  - u-order: pixel p < 3072: T = p//1024, par8 = p%8, i = (p%1024)//8,
    u = 1024*T + 128*par8 + i. Tail (q = p-3072): u = 3072 + 32*(q%2) + q//2.
  - 4 images are in flight at SBUF partition bases {0,32,64,96}; their
    matmuls are issued round-robin so consecutive MMs hit different PE
    row-groups (LDWEIGHTS overlaps in-flight MATMULs; bf16 streams at
    1 cyc/row vs fp32's 4).
  - Each PSUM bank [128, 512] f32 holds 1024 pixels of one image (8 MMs,
    par8 = 0..7 into disjoint 64-col slices). DVE/ACT copy-cast each bank
    to a bf16 staging buffer.
  - Per 8-image group: ONE out-DMA covering all full tiles (1 KiB
    contiguous DRAM runs: partition q of bank T = pixels 1024T+8q..+7)
    plus one small tail DMA. Output DRAM is bf16; host upcasts to f32.
"""

import numpy as np
import ml_dtypes

import concourse.bass as bass
import concourse.mybir as mybir
from concourse import bacc
from concourse.tile import TileContext
from concourse.bass_utils import run_bass_kernel_spmd

F32 = mybir.dt.float32
BF16 = mybir.dt.bfloat16
NP_BF16 = np.dtype(ml_dtypes.bfloat16)

N_CORES = 8
IMGS = 128            # images per core (2 n x 64 c)
NPIX = 56 * 56        # 3136
KDIM = 10             # 9 taps + ones (bias) row
TAIL_PIX = 64
GROUP_IMGS = 8        # images staged per out-DMA group
IMG_COLS = 3 * 512    # stage cols per image (3 banks x 512)
TAIL_BASE = GROUP_IMGS * IMG_COLS
STAGE_COLS = TAIL_BASE + GROUP_IMGS * 128


def build_nc(imgs=IMGS, group_imgs=GROUP_IMGS, loop_repeat=1, psum_bufs=7,
             n_sh=3, stage_bufs=4, psumt_bufs=1, in_engines="sg",
             out_split="img", unroll=1):
    n_groups = imgs // group_imgs
    tail_base = group_imgs * IMG_COLS
    stage_cols = tail_base + group_imgs * 128
    assert group_imgs % 4 == 0

    nc = bacc.Bacc("TRN2", target_bir_lowering=False, debug=False)
    sd = nc.dram_tensor("s", [imgs, KDIM, NPIX], BF16, kind="ExternalInput")
    # tail taps for all images, packed at partition base 0 (tile_position on
    # M=32 matmuls faults on HW, so tails must run without it)
    td = nc.dram_tensor("st", [KDIM, imgs * TAIL_PIX], BF16,
                        kind="ExternalInput")
    wd = nc.dram_tensor("w", [128, 64], BF16, kind="ExternalInput")
    out = nc.dram_tensor("out", [imgs * NPIX * 64], BF16, kind="ExternalOutput")

    with TileContext(nc) as tc:
        with (
            tc.tile_pool(name="const", bufs=1) as constp,
            tc.tile_pool(name="shift", bufs=n_sh) as shiftp,
            tc.tile_pool(name="stage", bufs=stage_bufs) as stagep,
            tc.tile_pool(name="psum", bufs=psum_bufs, space="PSUM") as psump,
            tc.tile_pool(name="psumt", bufs=psumt_bufs, space="PSUM") as psumt,
        ):
            wt = constp.tile([128, 64], BF16)
            nc.sync.dma_start(out=wt[:, :], in_=wd[:, :])
            stt = constp.tile([128, imgs * TAIL_PIX], BF16)
            nc.sync.dma_start(out=stt[0:KDIM, :], in_=td[:, :])

            def body():
                copy_idx = 0
                for g in range(n_groups):
                    stage = stagep.tile([128, stage_cols], BF16, tag="stage")
                    for bt in range(group_imgs // 4):
                        sh = shiftp.tile([128, NPIX], BF16, tag="sh")
                        ptail = psumt.tile([128, 512], F32, tag="ptail")
                        engs = {"sg": (nc.scalar, nc.gpsimd),
                                "ss": (nc.scalar, nc.scalar),
                                "ys": (nc.sync, nc.scalar),
                                "yg": (nc.sync, nc.gpsimd),
                                "sgy": (nc.scalar, nc.gpsimd, nc.sync)}[
                                    in_engines]
                        for j in range(4):
                            img = g * group_imgs + 4 * bt + j
                            eng = engs[img % len(engs)]
                            eng.dma_start(out=sh[32 * j:32 * j + KDIM, :],
                                          in_=sd[img])
                        pf = [[None] * 3 for _ in range(4)]
                        # 24 full MMs per image, issued round-robin over the
                        # 4 partition bases so LDWEIGHTS overlaps MATMULs.
                        for r in range(24):
                            T, par8 = divmod(r, 8)
                            for j in range(4):
                                if par8 == 0:
                                    pfull = psump.tile([128, 512], F32,
                                                       tag="pfull")
                                    pf[j][T] = pfull
                                lhsT = sh[32 * j:32 * j + KDIM,
                                          1024 * T + 128 * par8:
                                          1024 * T + 128 * (par8 + 1)]
                                nc.tensor.matmul(
                                    out=pf[j][T][:, 64 * par8:64 * (par8 + 1)],
                                    lhsT=lhsT, rhs=wt[32 * j:32 * j + KDIM, :],
                                    start=True, stop=True,
                                    tile_position=(32 * j, 0),
                                )
                                if par8 == 7:
                                    li = 4 * bt + j
                                    dst = stage[:, li * IMG_COLS + 512 * T:
                                                li * IMG_COLS + 512 * (T + 1)]
                                    if copy_idx % 2 == 0:
                                        nc.vector.tensor_copy(dst, pf[j][T][:, :])
                                    else:
                                        nc.scalar.copy(dst, pf[j][T][:, :])
                                    copy_idx += 1
                        # tail: 64 leftover pixels per image; 4 images share
                        # one PSUM bank (M=32 MMs, par in {0,1}); reads the
                        # base-0 packed tail tile, so no tile_position.
                        for j in range(4):
                            img = g * group_imgs + 4 * bt + j
                            for par in range(2):
                                lhsT = stt[0:KDIM,
                                           TAIL_PIX * img + 32 * par:
                                           TAIL_PIX * img + 32 * (par + 1)]
                                nc.tensor.matmul(
                                    out=ptail[0:32, 128 * j + 64 * par:
                                              128 * j + 64 * (par + 1)],
                                    lhsT=lhsT, rhs=wt[0:KDIM, :],
                                    start=True, stop=True,
                                )
                        dst = stage[0:32, tail_base + 512 * bt:
                                    tail_base + 512 * (bt + 1)]
                        if copy_idx % 2 == 0:
                            nc.vector.tensor_copy(dst, ptail[0:32, :])
                        else:
                            nc.scalar.copy(dst, ptail[0:32, :])
                        copy_idx += 1
                    # ---- group out-DMAs (3 KiB contiguous DRAM runs) ----
                    base = g * group_imgs * NPIX * 64
                    if out_split == "group":
                        out_full = bass.AP(
                            out, base,
                            [[IMG_COLS, 128], [NPIX * 64, group_imgs],
                             [1, IMG_COLS]],
                        )
                        nc.sync.dma_start(
                            out=out_full,
                            in_=stage[:, 0:group_imgs * IMG_COLS])
                    else:
                        for li in range(group_imgs):
                            out_full = bass.AP(
                                out, base + li * NPIX * 64,
                                [[IMG_COLS, 128], [1, IMG_COLS]],
                            )
                            nc.sync.dma_start(
                                out=out_full,
                                in_=stage[:, li * IMG_COLS:
                                          (li + 1) * IMG_COLS])
                    out_tail = bass.AP(
                        out, base + 3072 * 64,
                        [[128, 32], [NPIX * 64, group_imgs], [1, 128]],
                    )
                    nc.sync.dma_start(
                        out=out_tail,
                        in_=stage[0:32, tail_base:tail_base + group_imgs * 128])

            if loop_repeat > 1:
                with tc.For_i(0, loop_repeat):
                    for _ in range(unroll):
                        body()
            else:
                for _ in range(unroll):
                    body()
    nc.compile()
    return nc


_CACHE = {}


def _get_nc(imgs=IMGS, group_imgs=GROUP_IMGS, loop_repeat=1, unroll=1):
    key = (imgs, group_imgs, loop_repeat, unroll)
    if key not in _CACHE:
        _CACHE[key] = build_nc(imgs, group_imgs, loop_repeat=loop_repeat,
                               unroll=unroll)
    return _CACHE[key]


def _u_perm():
    """p_of_u[u] = pixel index stored at u-position u.

    Full tiles: pixel p = 24q + j (q: partition, j = 8T + par8: matmul index)
    lives at u = 128j + q, so partition q's 24 pixel outputs are contiguous
    in DRAM (3 KiB bf16 runs)."""
    p = np.arange(NPIX - TAIL_PIX)
    q, j = np.divmod(p, 24)
    u_full = 128 * j + q
    t = np.arange(TAIL_PIX)
    u_tail = (NPIX - TAIL_PIX) + 32 * (t % 2) + t // 2
    u_of_p = np.concatenate([u_full, u_tail])
    p_of_u = np.empty(NPIX, dtype=np.int64)
    p_of_u[u_of_p] = np.arange(NPIX)
    return p_of_u


_P_OF_U = _u_perm()


def _prep_inputs(x, W, b):
    x = np.ascontiguousarray(np.asarray(x, dtype=np.float32))
    W = np.ascontiguousarray(np.asarray(W, dtype=np.float32))
    b = np.ascontiguousarray(np.asarray(b, dtype=np.float32))
    N, C, H, Wd = x.shape
    nimg = N * C
    xpad = np.zeros((nimg, 58, 58), dtype=np.float32)
    xpad[:, 1:57, 1:57] = x.reshape(nimg, H, Wd)
    # S[img, d, p] = xpad[img, p//56 + d//3, p%56 + d%3]; d=9 -> ones
    S = np.empty((nimg, KDIM, NPIX), dtype=np.float32)
    for d in range(9):
        di, dj = divmod(d, 3)
        S[:, d, :] = xpad[:, di:di + 56, dj:dj + 56].reshape(nimg, NPIX)
    S[:, 9, :] = 1.0
    S = S[:, :, _P_OF_U]                      # u-order
    S = np.ascontiguousarray(
        S.reshape(N_CORES, nimg // N_CORES, KDIM, NPIX)).astype(NP_BF16)
    # tail taps packed [KDIM, imgs*64] per core (u-order tail block)
    ST = np.ascontiguousarray(
        S[:, :, :, NPIX - TAIL_PIX:].transpose(0, 2, 1, 3).reshape(
            N_CORES, KDIM, (nimg // N_CORES) * TAIL_PIX))
    wb = np.concatenate([W, b[None, :]], axis=0)          # [10, 64]
    w_rep = np.zeros((128, 64), dtype=np.float32)
    for j in range(4):
        w_rep[32 * j:32 * j + KDIM] = wb
    w_rep = w_rep.astype(NP_BF16)
    in_maps = [{"s": S[i], "st": ST[i], "w": w_rep} for i in range(N_CORES)]
    return in_maps, N, C


def run(x, W, b, trace=False, **kw):
    in_maps, N, C = _prep_inputs(x, W, b)
    nc = _get_nc()
    res = run_bass_kernel_spmd(
        nc, in_maps, core_ids=list(range(N_CORES)), trace=trace, **kw
    )
    outs = [
        np.asarray(res.results[i]["out"]).reshape(N // N_CORES, C, NPIX, 64)
        for i in range(N_CORES)
    ]
    full = np.concatenate(outs, axis=0).astype(np.float32)
    return full, res


def kernel(x, W, b):
    full, _ = run(x, W, b, trace=False)
    return full


# ---------------------------------------------------------------------------
# benchmarking helpers (not used by the grading harness)
# ---------------------------------------------------------------------------

def make_bench_fn(nc, in_maps):
    """jit-compiled PJRT executor over 8 cores with device-resident inputs
    and donated output chaining (no host transfers on the timed path)."""
    import jax
    from jax.sharding import Mesh, PartitionSpec, NamedSharding
    from jax.experimental.shard_map import shard_map
    from concourse import bass2jax as b2j

    b2j.install_neuronx_cc_hook()
    partition_name = (
        nc.partition_id_tensor.name if nc.partition_id_tensor else None
    )
    in_names, out_names, out_avals = [], [], []
    for alloc in nc.m.functions[0].allocations:
        if not isinstance(alloc, mybir.MemoryLocationSet):
            continue
        name = alloc.memorylocations[0].name
        if alloc.kind == "ExternalInput":
            if name != partition_name:
                in_names.append(name)
        elif alloc.kind == "ExternalOutput":
            out_names.append(name)
            out_avals.append(jax.core.ShapedArray(
                tuple(alloc.tensor_shape), mybir.dt.np(alloc.dtype)))
    n_params = len(in_names)
    n_outs = len(out_avals)
    all_names = in_names + out_names
    if partition_name is not None:
        all_names = all_names + [partition_name]

    def _body(*args):
        operands = list(args)
        if partition_name is not None:
            operands.append(b2j.partition_id_tensor())
        return tuple(b2j._bass_exec_p.bind(
            *operands, out_avals=tuple(out_avals), in_names=tuple(all_names),
            out_names=tuple(out_names), lowering_input_output_aliases=(),
            sim_require_finite=True, sim_require_nnan=True, nc=nc))

    devices = jax.devices()[:N_CORES]
    mesh = Mesh(np.asarray(devices), ("core",))
    fn = jax.jit(
        shard_map(_body, mesh=mesh,
                  in_specs=(PartitionSpec("core"),) * (n_params + n_outs),
                  out_specs=(PartitionSpec("core"),) * n_outs,
                  check_rep=False),
        donate_argnums=tuple(range(n_params, n_params + n_outs)),
        keep_unused=True)
    sh = NamedSharding(mesh, PartitionSpec("core"))
    concat_in = [np.concatenate([np.asarray(m[nm]) for m in in_maps], axis=0)
                 for nm in in_names]
    dev_in = [jax.device_put(a, sh) for a in concat_in]
    outs = tuple(jax.device_put(
        np.zeros((N_CORES * a.shape[0], *a.shape[1:]), a.dtype), sh)
        for a in out_avals)
    return fn, dev_in, outs


def bench(x, W, b, loop_repeat=32, unroll=4, reps=7, n_lo=2, n_hi=10):
    """Estimate steady-state per-workload execution time.

    The benchmark NEFF runs the complete workload ``loop_repeat * unroll``
    times (``unroll`` python-unrolled copies inside a ``For_i`` hardware
    loop; identical instructions and addresses each iteration).  We time
    asynchronously-enqueued batches of n_lo and n_hi executions and use the
    marginal slope (t_hi - t_lo) / (n_hi - n_lo) to cancel the constant
    axon-tunnel round-trip; dividing by the repeat count gives the
    per-workload time.  Reports the median over ``reps`` slope measurements.

    Falls back to smaller repeat configs if a transient device fault kills
    the preferred benchmark NEFF.
    """
    for lr, ur in [(loop_repeat, unroll), (loop_repeat, unroll),
                   (2 * loop_repeat, 1), (1, 1)]:
        try:
            return _bench_inner(x, W, b, lr, ur, reps, n_lo, n_hi)
        except Exception as e:  # noqa: BLE001 - retry on device faults
            last = e
    raise last


def _bench_inner(x, W, b, loop_repeat, unroll, reps, n_lo, n_hi):
    import time
    import jax

    in_maps, _, _ = _prep_inputs(x, W, b)
    nc = _get_nc(loop_repeat=loop_repeat, unroll=unroll)
    fn, dev_in, outs = make_bench_fn(nc, in_maps)

    # warmup (compile + 2 blocking executions)
    for _ in range(2):
        outs = fn(*dev_in, *outs)
        jax.block_until_ready(outs)

    def timed_batch(n):
        nonlocal outs
        t0 = time.perf_counter()
        for _ in range(n):
            outs = fn(*dev_in, *outs)
        jax.block_until_ready(outs)
        return time.perf_counter() - t0

    slopes = []
    for _ in range(reps):
        t_lo = timed_batch(n_lo)
        t_hi = timed_batch(n_hi)
        slopes.append((t_hi - t_lo) / (n_hi - n_lo))
    slopes.sort()
    med_slope = slopes[len(slopes) // 2]
    per_workload = med_slope / (loop_repeat * unroll)
    return per_workload, {
        "slopes_ms": [s * 1e3 for s in slopes],
        "per_exec_ms": med_slope * 1e3,
        "loop_repeat": loop_repeat * unroll,
    }


def timeline(out_path=None, imgs=16, group_imgs=GROUP_IMGS):
    """Cost-model simulation of a reduced-size variant; returns modeled ns."""
    from concourse.timeline_sim import TimelineSim
    nc = build_nc(imgs=imgs, group_imgs=group_imgs)
    ts = TimelineSim(nc, trace=False)
    return ts.simulate()


# revision 34
# speedup vs baseline: 1.0654x; 1.0654x over previous
"""Trainium2 Bass kernel for MDMLPPatch (3x3 unfold + per-channel linear 9->64).

out[n,c,p,e] = sum_d patches[n,c,p,d] * W[d,e] + b[e]
x: [16,64,56,56] f32, W: [9,64] f32, b: [64] f32 -> out: [16,64,3136,64] f32

Sharding: data-parallel over batch N: 16 n / 8 cores = 2 n per core.
Each core processes 128 independent 56x56 images (2 n x 64 c).

bf16 design (rel tol is 2e-2; bf16 keeps rel err ~5e-3):
  - Host ships S[img, d, u] = unfolded patches in u-order, bf16. d=0..8 are
    the 9 taps, d=9 is all-ones so the K=10 matmul contraction adds the bias.
  - u-order: pixel p < 3072: T = p//1024, par8 = p%8, i = (p%1024)//8,
    u = 1024*T + 128*par8 + i. Tail (q = p-3072): u = 3072 + 32*(q%2) + q//2.
  - 4 images are in flight at SBUF partition bases {0,32,64,96}; their
    matmuls are issued round-robin so consecutive MMs hit different PE
    row-groups (LDWEIGHTS overlaps in-flight MATMULs; bf16 streams at
    1 cyc/row vs fp32's 4).
  - Each PSUM bank [128, 512] f32 holds 1024 pixels of one image (8 MMs,
    par8 = 0..7 into disjoint 64-col slices). DVE/ACT copy-cast each bank
    to a bf16 staging buffer.
  - Per 8-image group: ONE out-DMA covering all full tiles (1 KiB
    contiguous DRAM runs: partition q of bank T = pixels 1024T+8q..+7)
    plus one small tail DMA. Output DRAM is bf16; host upcasts to f32.
"""

import numpy as np
import ml_dtypes

import concourse.bass as bass
import concourse.mybir as mybir
from concourse import bacc
from concourse.tile import TileContext
from concourse.bass_utils import run_bass_kernel_spmd

F32 = mybir.dt.float32
BF16 = mybir.dt.bfloat16
NP_BF16 = np.dtype(ml_dtypes.bfloat16)

N_CORES = 8
IMGS = 128            # images per core (2 n x 64 c)
NPIX = 56 * 56        # 3136
KDIM = 10             # 9 taps + ones (bias) row
TAIL_PIX = 64
GROUP_IMGS = 8        # images staged per out-DMA group
IMG_COLS = 3 * 512    # stage cols per image (3 banks x 512)
TAIL_BASE = GROUP_IMGS * IMG_COLS
STAGE_COLS = TAIL_BASE + GROUP_IMGS * 128


def build_nc(imgs=IMGS, group_imgs=GROUP_IMGS, loop_repeat=1, psum_bufs=7,
             n_sh=5, stage_bufs=4, psumt_bufs=1, in_engines="sg",
             out_split="img", unroll=1, copy_any=True):
    n_groups = imgs // group_imgs
    tail_base = group_imgs * IMG_COLS
    stage_cols = tail_base + group_imgs * 128
    assert group_imgs % 4 == 0

    nc = bacc.Bacc("TRN2", target_bir_lowering=False, debug=False)
    sd = nc.dram_tensor("s", [imgs, KDIM, NPIX], BF16, kind="ExternalInput")
    # tail taps for all images, packed at partition base 0 (tile_position on
    # M=32 matmuls faults on HW, so tails must run without it)
    td = nc.dram_tensor("st", [KDIM, imgs * TAIL_PIX], BF16,
                        kind="ExternalInput")
    wd = nc.dram_tensor("w", [128, 64], BF16, kind="ExternalInput")
    out = nc.dram_tensor("out", [imgs * NPIX * 64], BF16, kind="ExternalOutput")

    with TileContext(nc) as tc:
        with (
            tc.tile_pool(name="const", bufs=1) as constp,
            tc.tile_pool(name="shift", bufs=n_sh) as shiftp,
            tc.tile_pool(name="stage", bufs=stage_bufs) as stagep,
            tc.tile_pool(name="psum", bufs=psum_bufs, space="PSUM") as psump,
            tc.tile_pool(name="psumt", bufs=psumt_bufs, space="PSUM") as psumt,
        ):
            wt = constp.tile([128, 64], BF16)
            nc.sync.dma_start(out=wt[:, :], in_=wd[:, :])
            stt = constp.tile([128, imgs * TAIL_PIX], BF16)
            nc.sync.dma_start(out=stt[0:KDIM, :], in_=td[:, :])

            def body():
                copy_idx = 0
                for g in range(n_groups):
                    stage = stagep.tile([128, stage_cols], BF16, tag="stage")
                    for bt in range(group_imgs // 4):
                        sh = shiftp.tile([128, NPIX], BF16, tag="sh")
                        ptail = psumt.tile([128, 512], F32, tag="ptail")
                        engs = {"sg": (nc.scalar, nc.gpsimd),
                                "ss": (nc.scalar, nc.scalar),
                                "ys": (nc.sync, nc.scalar),
                                "yg": (nc.sync, nc.gpsimd),
                                "sgy": (nc.scalar, nc.gpsimd, nc.sync)}[
                                    in_engines]
                        for j in range(4):
                            img = g * group_imgs + 4 * bt + j
                            eng = engs[img % len(engs)]
                            eng.dma_start(out=sh[32 * j:32 * j + KDIM, :],
                                          in_=sd[img])
                        pf = [[None] * 3 for _ in range(4)]
                        # 24 full MMs per image, issued round-robin over the
                        # 4 partition bases so LDWEIGHTS overlaps MATMULs.
                        for r in range(24):
                            T, par8 = divmod(r, 8)
                            for j in range(4):
                                if par8 == 0:
                                    pfull = psump.tile([128, 512], F32,
                                                       tag="pfull")
                                    pf[j][T] = pfull
                                lhsT = sh[32 * j:32 * j + KDIM,
                                          1024 * T + 128 * par8:
                                          1024 * T + 128 * (par8 + 1)]
                                nc.tensor.matmul(
                                    out=pf[j][T][:, 64 * par8:64 * (par8 + 1)],
                                    lhsT=lhsT, rhs=wt[32 * j:32 * j + KDIM, :],
                                    start=True, stop=True,
                                    tile_position=(32 * j, 0),
                                )
                                if par8 == 7:
                                    li = 4 * bt + j
                                    dst = stage[:, li * IMG_COLS + 512 * T:
                                                li * IMG_COLS + 512 * (T + 1)]
                                    if copy_any:
                                        nc.any.tensor_copy(dst, pf[j][T][:, :])
                                    elif copy_idx % 2 == 0:
                                        nc.vector.tensor_copy(dst, pf[j][T][:, :])
                                    else:
                                        nc.scalar.copy(dst, pf[j][T][:, :])
                                    copy_idx += 1
                        # tail: 64 leftover pixels per image; 4 images share
                        # one PSUM bank (M=32 MMs, par in {0,1}); reads the
                        # base-0 packed tail tile, so no tile_position.
                        for j in range(4):
                            img = g * group_imgs + 4 * bt + j
                            for par in range(2):
                                lhsT = stt[0:KDIM,
                                           TAIL_PIX * img + 32 * par:
                                           TAIL_PIX * img + 32 * (par + 1)]
                                nc.tensor.matmul(
                                    out=ptail[0:32, 128 * j + 64 * par:
                                              128 * j + 64 * (par + 1)],
                                    lhsT=lhsT, rhs=wt[0:KDIM, :],
                                    start=True, stop=True,
                                )
                        dst = stage[0:32, tail_base + 512 * bt:
                                    tail_base + 512 * (bt + 1)]
                        if copy_any:
                            nc.any.tensor_copy(dst, ptail[0:32, :])
                        elif copy_idx % 2 == 0:
                            nc.vector.tensor_copy(dst, ptail[0:32, :])
                        else:
                            nc.scalar.copy(dst, ptail[0:32, :])
                        copy_idx += 1
                    # ---- group out-DMAs (3 KiB contiguous DRAM runs) ----
                    base = g * group_imgs * NPIX * 64
                    if out_split == "group":
                        out_full = bass.AP(
                            out, base,
                            [[IMG_COLS, 128], [NPIX * 64, group_imgs],
                             [1, IMG_COLS]],
                        )
                        nc.sync.dma_start(
                            out=out_full,
                            in_=stage[:, 0:group_imgs * IMG_COLS])
                    else:
                        for li in range(group_imgs):
                            out_full = bass.AP(
                                out, base + li * NPIX * 64,
                                [[IMG_COLS, 128], [1, IMG_COLS]],
                            )
                            nc.sync.dma_start(
                                out=out_full,
                                in_=stage[:, li * IMG_COLS:
                                          (li + 1) * IMG_COLS])
                    out_tail = bass.AP(
                        out, base + 3072 * 64,
                        [[128, 32], [NPIX * 64, group_imgs], [1, 128]],
                    )
                    nc.sync.dma_start(
                        out=out_tail,
                        in_=stage[0:32, tail_base:tail_base + group_imgs * 128])

            if loop_repeat > 1:
                with tc.For_i(0, loop_repeat):
                    for _ in range(unroll):
                        body()
            else:
                for _ in range(unroll):
                    body()
    nc.compile()
    return nc


_CACHE = {}


def _get_nc(imgs=IMGS, group_imgs=GROUP_IMGS, loop_repeat=1, unroll=1):
    key = (imgs, group_imgs, loop_repeat, unroll)
    if key not in _CACHE:
        _CACHE[key] = build_nc(imgs, group_imgs, loop_repeat=loop_repeat,
                               unroll=unroll)
    return _CACHE[key]


def _u_perm():
    """p_of_u[u] = pixel index stored at u-position u.

    Full tiles: pixel p = 24q + j (q: partition, j = 8T + par8: matmul index)
    lives at u = 128j + q, so partition q's 24 pixel outputs are contiguous
    in DRAM (3 KiB bf16 runs)."""
    p = np.arange(NPIX - TAIL_PIX)
    q, j = np.divmod(p, 24)
    u_full = 128 * j + q
    t = np.arange(TAIL_PIX)
    u_tail = (NPIX - TAIL_PIX) + 32 * (t % 2) + t // 2
    u_of_p = np.concatenate([u_full, u_tail])
    p_of_u = np.empty(NPIX, dtype=np.int64)
    p_of_u[u_of_p] = np.arange(NPIX)
    return p_of_u


_P_OF_U = _u_perm()


def _prep_inputs(x, W, b):
    x = np.ascontiguousarray(np.asarray(x, dtype=np.float32))
    W = np.ascontiguousarray(np.asarray(W, dtype=np.float32))
    b = np.ascontiguousarray(np.asarray(b, dtype=np.float32))
    N, C, H, Wd = x.shape
    nimg = N * C
    xpad = np.zeros((nimg, 58, 58), dtype=np.float32)
    xpad[:, 1:57, 1:57] = x.reshape(nimg, H, Wd)
    # S[img, d, p] = xpad[img, p//56 + d//3, p%56 + d%3]; d=9 -> ones
    S = np.empty((nimg, KDIM, NPIX), dtype=np.float32)
    for d in range(9):
        di, dj = divmod(d, 3)
        S[:, d, :] = xpad[:, di:di + 56, dj:dj + 56].reshape(nimg, NPIX)
    S[:, 9, :] = 1.0
    S = S[:, :, _P_OF_U]                      # u-order
    S = np.ascontiguousarray(
        S.reshape(N_CORES, nimg // N_CORES, KDIM, NPIX)).astype(NP_BF16)
    # tail taps packed [KDIM, imgs*64] per core (u-order tail block)
    ST = np.ascontiguousarray(
        S[:, :, :, NPIX - TAIL_PIX:].transpose(0, 2, 1, 3).reshape(
            N_CORES, KDIM, (nimg // N_CORES) * TAIL_PIX))
    wb = np.concatenate([W, b[None, :]], axis=0)          # [10, 64]
    w_rep = np.zeros((128, 64), dtype=np.float32)
    for j in range(4):
        w_rep[32 * j:32 * j + KDIM] = wb
    w_rep = w_rep.astype(NP_BF16)
    in_maps = [{"s": S[i], "st": ST[i], "w": w_rep} for i in range(N_CORES)]
    return in_maps, N, C


def run(x, W, b, trace=False, **kw):
    in_maps, N, C = _prep_inputs(x, W, b)
    nc = _get_nc()
    res = run_bass_kernel_spmd(
        nc, in_maps, core_ids=list(range(N_CORES)), trace=trace, **kw
    )
    outs = [
        np.asarray(res.results[i]["out"]).reshape(N // N_CORES, C, NPIX, 64)
        for i in range(N_CORES)
    ]
    full = np.concatenate(outs, axis=0).astype(np.float32)
    return full, res


def kernel(x, W, b):
    full, _ = run(x, W, b, trace=False)
    return full


# ---------------------------------------------------------------------------
# benchmarking helpers (not used by the grading harness)
# ---------------------------------------------------------------------------

def make_bench_fn(nc, in_maps):
    """jit-compiled PJRT executor over 8 cores with device-resident inputs
    and donated output chaining (no host transfers on the timed path)."""
    import jax
    from jax.sharding import Mesh, PartitionSpec, NamedSharding
    from jax.experimental.shard_map import shard_map
    from concourse import bass2jax as b2j

    b2j.install_neuronx_cc_hook()
    partition_name = (
        nc.partition_id_tensor.name if nc.partition_id_tensor else None
    )
    in_names, out_names, out_avals = [], [], []
    for alloc in nc.m.functions[0].allocations:
        if not isinstance(alloc, mybir.MemoryLocationSet):
            continue
        name = alloc.memorylocations[0].name
        if alloc.kind == "ExternalInput":
            if name != partition_name:
                in_names.append(name)
        elif alloc.kind == "ExternalOutput":
            out_names.append(name)
            out_avals.append(jax.core.ShapedArray(
                tuple(alloc.tensor_shape), mybir.dt.np(alloc.dtype)))
    n_params = len(in_names)
    n_outs = len(out_avals)
    all_names = in_names + out_names
    if partition_name is not None:
        all_names = all_names + [partition_name]

    def _body(*args):
        operands = list(args)
        if partition_name is not None:
            operands.append(b2j.partition_id_tensor())
        return tuple(b2j._bass_exec_p.bind(
            *operands, out_avals=tuple(out_avals), in_names=tuple(all_names),
            out_names=tuple(out_names), lowering_input_output_aliases=(),
            sim_require_finite=True, sim_require_nnan=True, nc=nc))

    devices = jax.devices()[:N_CORES]
    mesh = Mesh(np.asarray(devices), ("core",))
    fn = jax.jit(
        shard_map(_body, mesh=mesh,
                  in_specs=(PartitionSpec("core"),) * (n_params + n_outs),
                  out_specs=(PartitionSpec("core"),) * n_outs,
                  check_rep=False),
        donate_argnums=tuple(range(n_params, n_params + n_outs)),
        keep_unused=True)
    sh = NamedSharding(mesh, PartitionSpec("core"))
    concat_in = [np.concatenate([np.asarray(m[nm]) for m in in_maps], axis=0)
                 for nm in in_names]
    dev_in = [jax.device_put(a, sh) for a in concat_in]
    outs = tuple(jax.device_put(
        np.zeros((N_CORES * a.shape[0], *a.shape[1:]), a.dtype), sh)
        for a in out_avals)
    return fn, dev_in, outs


def bench(x, W, b, loop_repeat=32, unroll=4, reps=7, n_lo=2, n_hi=10):
    """Estimate steady-state per-workload execution time.

    The benchmark NEFF runs the complete workload ``loop_repeat * unroll``
    times (``unroll`` python-unrolled copies inside a ``For_i`` hardware
    loop; identical instructions and addresses each iteration).  We time
    asynchronously-enqueued batches of n_lo and n_hi executions and use the
    marginal slope (t_hi - t_lo) / (n_hi - n_lo) to cancel the constant
    axon-tunnel round-trip; dividing by the repeat count gives the
    per-workload time.  Reports the median over ``reps`` slope measurements.

    Falls back to smaller repeat configs if a transient device fault kills
    the preferred benchmark NEFF.
    """
    for lr, ur in [(loop_repeat, unroll), (loop_repeat, unroll),
                   (2 * loop_repeat, 1), (1, 1)]:
        try:
            return _bench_inner(x, W, b, lr, ur, reps, n_lo, n_hi)
        except Exception as e:  # noqa: BLE001 - retry on device faults
            last = e
    raise last


def _bench_inner(x, W, b, loop_repeat, unroll, reps, n_lo, n_hi):
    import time
    import jax

    in_maps, _, _ = _prep_inputs(x, W, b)
    nc = _get_nc(loop_repeat=loop_repeat, unroll=unroll)
    fn, dev_in, outs = make_bench_fn(nc, in_maps)

    # warmup (compile + 2 blocking executions)
    for _ in range(2):
        outs = fn(*dev_in, *outs)
        jax.block_until_ready(outs)

    def timed_batch(n):
        nonlocal outs
        t0 = time.perf_counter()
        for _ in range(n):
            outs = fn(*dev_in, *outs)
        jax.block_until_ready(outs)
        return time.perf_counter() - t0

    slopes = []
    for _ in range(reps):
        t_lo = timed_batch(n_lo)
        t_hi = timed_batch(n_hi)
        slopes.append((t_hi - t_lo) / (n_hi - n_lo))
    slopes.sort()
    med_slope = slopes[len(slopes) // 2]
    per_workload = med_slope / (loop_repeat * unroll)
    return per_workload, {
        "slopes_ms": [s * 1e3 for s in slopes],
        "per_exec_ms": med_slope * 1e3,
        "loop_repeat": loop_repeat * unroll,
    }


def timeline(out_path=None, imgs=16, group_imgs=GROUP_IMGS):
    """Cost-model simulation of a reduced-size variant; returns modeled ns."""
    from concourse.timeline_sim import TimelineSim
    nc = build_nc(imgs=imgs, group_imgs=group_imgs)
    ts = TimelineSim(nc, trace=False)
    return ts.simulate()
